# revision 25
# baseline (speedup 1.0000x reference)
"""Trainium2 Bass kernel: transformer block (biased attention + residual).

Reference math (B=4, S=1024, H=1024, NH=16, DK=64):
    q = x_q @ Wq.T ; k = x_kv @ Wk.T ; v = x_kv @ Wv.T   (per-head reshape)
    scores = q k^T / sqrt(DK) + bias ; attn = softmax(scores)
    out = x_q + (attn v reshaped) @ Wo.T

Sharding: 8 cores = 4 batches x 2 head-groups (8 heads each). Each core
computes its (batch, head-group) slice; the host sums the two head-group
partial outputs per batch and adds the residual.

Per-core dataflow (all matmul inputs bf16, PSUM accumulation fp32):
    qT/kT = W_g x^T           (head_dim on partitions, seq on free)
    v     = x_kv @ Wv_g.T     (seq on partitions), padded with a ones column
    scoresT[k,q] = k_h q_h^T (pair-packed K=64 row tiles)
                   + I^T biasT (bias added via identity matmul into PSUM)
    expT  = exp(scoresT)      (no max subtraction needed: |scores| <~ 8)
    avT   = v_aug^T expT      -> rows 0..63 = attn out^T, row 64 = denom
    aoT   = avT[0:64] * broadcast(1/denom)   (K=1 matmul broadcast)
    yT    = Wo_g^T-contraction of aoT        (partial, fp32 out)
"""

import sys

import numpy as np

for _p in ("/opt/trn_rl_repo",):
    if _p not in sys.path:
        sys.path.append(_p)

B, S, H, NH = 4, 1024, 1024, 16
DK = 64
P = 128
NH_L = 8            # heads per core
JL = NH_L * DK      # 512 local head dims per core
FT = H // P         # 8 contraction tiles for projections
TT = S // P         # 8 seq tiles
JC = JL // P        # 4 local head-dim chunks of 128
QF = 512            # matmul moving free dim (one PSUM bank of fp32)
QC = S // QF        # 2 q chunks
N_CORES = 8


def _split_waits(nc, max_waits=1):
    """This walrus build rejects instructions carrying more than ~1 sem
    wait ("Too many sync wait commands" in setupSyncWait). Hoist surplus
    waits onto same-engine NoOps spliced immediately before the carrying
    instruction — same engine position, so semantics are unchanged."""
    import bass_rust
    import concourse.mybir as mybir

    n = 0
    for f in nc.m.functions:
        for bb in f.blocks:
            new_insts = []
            for inst in bb.instructions:
                si = inst.sync_info
                waits = list(si.on_wait) if si and si.on_wait else []
                if len(waits) > max_waits:
                    keep = waits[:max_waits]
                    extra = waits[max_waits:]
                    for i in range(0, len(extra), max_waits):
                        nop = mybir.InstNoOp(name=f"WSPLIT-{n}", ins=[], outs=[])
                        n += 1
                        nop.engine = inst.engine
                        nop.bass_nofuse = False
                        nop.debug = inst.debug
                        nop.sync_info = bass_rust.SyncInfo(
                            on_wait=extra[i : i + max_waits], on_update=[]
                        )
                        new_insts.append(nop)
                    si.on_wait = keep
                    inst.sync_info = si
                new_insts.append(inst)
            bb.instructions[:] = new_insts


_prog = None


def _build():
    global _prog
    if _prog is not None:
        return _prog

    import concourse.bass as bass
    import concourse.mybir as mybir
    import concourse.tile as tile
    from concourse.masks import make_identity

    f32 = mybir.dt.float32
    bf16 = mybir.dt.bfloat16
    EXP = mybir.ActivationFunctionType.Exp
    MULT = mybir.AluOpType.mult

    nc = bass.Bass()
    xqT_d = nc.declare_dram_parameter("xqT", [H, S], bf16, isOutput=False)
    xkvT_d = nc.declare_dram_parameter("xkvT", [H, S], bf16, isOutput=False)
    wqT_d = nc.declare_dram_parameter("wqT", [H, JL], bf16, isOutput=False)
    wkT_d = nc.declare_dram_parameter("wkT", [H, JL], bf16, isOutput=False)
    wvT_d = nc.declare_dram_parameter("wvT", [H, JL], bf16, isOutput=False)
    woT_d = nc.declare_dram_parameter("woT", [JL, H], bf16, isOutput=False)
    biasT_d = nc.declare_dram_parameter("biasT", [NH_L, S, S], bf16, isOutput=False)
    yT_d = nc.declare_dram_parameter("yT", [H, S], f32, isOutput=True)

    with tile.TileContext(nc) as tc:
        with (
            tc.tile_pool(name="singles", bufs=1) as singles,
            tc.tile_pool(name="biasp", bufs=3) as biasp,
            tc.tile_pool(name="expp", bufs=18) as expp,
            tc.tile_pool(name="smallp", bufs=3) as smallp,
            tc.tile_pool(name="outp", bufs=3) as outp,
            tc.tile_pool(name="ps_s", bufs=4, space="PSUM") as ps_s,
            tc.tile_pool(name="ps_mm", bufs=2, space="PSUM") as ps_mm,
            tc.tile_pool(name="ps_av", bufs=2, space="PSUM") as ps_av,
        ):
            xq_sb = singles.tile([P, FT, S], bf16)
            xkv_sb = singles.tile([P, FT, S], bf16)
            wq_sb = singles.tile([P, FT, JL], bf16)
            wk_sb = singles.tile([P, FT, JL], bf16)
            wv_sb = singles.tile([P, FT, JL], bf16)
            wo_sb = singles.tile([P, JC, H], bf16)
            qT_sb = singles.tile([P, JC, S], bf16)
            kT_sb = singles.tile([P, JC, S], bf16)
            v_sb = singles.tile([P, TT, NH_L, DK + 1], bf16)
            aoT_sb = singles.tile([P, JC, S], bf16)
            ident = singles.tile([P, P], bf16)
            ones64 = singles.tile([1, DK], bf16)

            make_identity(nc, ident)
            nc.vector.memset(ones64, 1.0)
            nc.vector.memset(v_sb[:, :, :, DK : DK + 1], 1.0)

            def load2(sb, dr, cols=None):
                drr = dr.rearrange("(n p) j -> p n j", p=P)
                for f2 in range(FT // 2):
                    s = slice(2 * f2, 2 * f2 + 2)
                    if cols is None:
                        nc.sync.dma_start(out=sb[:, s, :], in_=drr[:, s, :])
                    else:
                        nc.sync.dma_start(
                            out=sb[:, s, cols], in_=drr[:, s, cols]
                        )

            load2(wq_sb, wqT_d)
            load2(xq_sb, xqT_d, cols=slice(0, QF))
            load2(wk_sb, wkT_d)
            load2(xkv_sb, xkvT_d, cols=slice(0, QF))
            load2(xq_sb, xqT_d, cols=slice(QF, S))
            load2(xkv_sb, xkvT_d, cols=slice(QF, S))
            for hdt in range(JC):
                nc.sync.dma_start(
                    out=wo_sb[:, hdt, :], in_=woT_d[hdt * P : (hdt + 1) * P, :]
                )

            bias_pref = {}

            def bias_fetch(hp, mt):
                out = []
                for i in range(2):
                    h = 2 * hp + i
                    bt = biasp.tile(
                        [P, S], bf16, name=f"bias_{h}_{mt}", tag=f"bias{i}"
                    )
                    nc.sync.dma_start(
                        out=bt, in_=biasT_d[h, mt * P : (mt + 1) * P, :]
                    )
                    out.append(bt)
                return out

            for _mt in range(3):
                bias_pref[(0, _mt)] = bias_fetch(0, _mt)
            load2(wv_sb, wvT_d)

            # (HAM warm-up) ~30 back-to-back tiny matmuls while the input
            # DMAs land, so the PE clock is at 8/8 when real work starts.
            warm_ps = ps_mm.tile([P, P], f32, name="warm", tag="mm")
            for _ in range(140):
                nc.tensor.matmul(warm_ps, lhsT=ident, rhs=ident,
                                 start=True, stop=True, skip_group_check=True)

            exp_tiles = {}

            def proj_qk(jc):
                for nm, w_sb, x_sb, out_sb, scale in (
                    ("q", wq_sb, xq_sb, qT_sb, 0.125),
                    ("k", wk_sb, xkv_sb, kT_sb, None),
                ):
                    for tch in range(QC):
                        ps = ps_mm.tile(
                            [P, QF], f32, name=f"pj{nm}_{jc}_{tch}", tag="mm"
                        )
                        for ft in range(FT):
                            nc.tensor.matmul(
                                ps,
                                lhsT=w_sb[:, ft, jc * P : (jc + 1) * P],
                                rhs=x_sb[:, ft, tch * QF : (tch + 1) * QF],
                                start=(ft == 0),
                                stop=(ft == FT - 1),
                            )
                        dst = out_sb[:, jc, tch * QF : (tch + 1) * QF]
                        if scale is None:
                            nc.vector.tensor_copy(out=dst, in_=ps)
                        else:
                            nc.vector.tensor_scalar_mul(dst, ps, scale)

            def proj_v():
                for tt in range(TT):
                    ps = ps_mm.tile([P, QF], f32, name=f"pjv_{tt}", tag="mm")
                    for ft in range(FT):
                        nc.tensor.matmul(
                            ps,
                            lhsT=xkv_sb[:, ft, tt * P : (tt + 1) * P],
                            rhs=wv_sb[:, ft, :],
                            start=(ft == 0),
                            stop=(ft == FT - 1),
                        )
                    nc.vector.tensor_copy(
                        out=v_sb[:, tt, :, 0:DK],
                        in_=ps.rearrange("p (h d) -> p h d", h=NH_L),
                    )

            def scores_step(hp, mt):
                if (hp, mt) in bias_pref:
                    btiles = bias_pref.pop((hp, mt))
                else:
                    btiles = bias_fetch(hp, mt)
                ptiles = [[None] * QC for _ in range(2)]
                for qc in range(QC):
                    for i in range(2):
                        # bias matmul first: it depends only on the bias DMA,
                        # so it can run before qT/kT chunks are projected
                        ps = ps_s.tile(
                            [P, QF], f32, name=f"sc_{hp}_{mt}_{i}_{qc}", tag="sc"
                        )
                        nc.tensor.matmul(
                            ps,
                            lhsT=ident,
                            rhs=btiles[i][:, qc * QF : (qc + 1) * QF],
                            start=True,
                            stop=False,
                            skip_group_check=True,
                        )
                        ptiles[i][qc] = ps
                    for i in range(2):
                        jr = i * DK
                        nc.tensor.matmul(
                            ptiles[i][qc],
                            lhsT=kT_sb[jr : jr + DK, hp, mt * P : (mt + 1) * P],
                            rhs=qT_sb[jr : jr + DK, hp, qc * QF : (qc + 1) * QF],
                            start=False,
                            stop=True,
                            skip_group_check=True,
                        )
                for i in range(2):
                    h = 2 * hp + i
                    et = expp.tile([P, S], bf16, name=f"exp_{h}_{mt}", tag="exp")
                    exp_tiles[(h, mt)] = et
                    for qc in range(QC):
                        nc.scalar.activation(
                            out=et[:, qc * QF : (qc + 1) * QF],
                            in_=ptiles[i][qc],
                            func=EXP,
                        )

            av_tiles = {}
            rec_rows = {}

            def attn_v_A(h, qc):
                # attn@v accumulation + cheap reciprocal of the denominator
                # row (reshaped (1,512)->(128,4) by DMA so the DVE recip runs
                # at FD=4 instead of FD=512; back-DMA on gpsimd casts bf16).
                av = ps_av.tile([P, QF], f32, name=f"av_{h}_{qc}", tag="av")
                av_tiles[(h, qc)] = av
                for mt in range(TT):
                    nc.tensor.matmul(
                        av[0 : DK + 1, :],
                        lhsT=v_sb[:, mt, h, :],
                        rhs=exp_tiles[(h, mt)][:, qc * QF : (qc + 1) * QF],
                        start=(mt == 0),
                        stop=(mt == TT - 1),
                    )
                den = smallp.tile([1, QF], f32, name=f"den_{h}_{qc}", tag="den")
                nc.vector.tensor_copy(out=den, in_=av[DK : DK + 1, :])
                den_r = smallp.tile([P, QF // P], f32, name=f"denr_{h}_{qc}", tag="denr")
                nc.sync.dma_start(out=den_r, in_=den)
                rec_r = smallp.tile([P, QF // P], f32, name=f"recr_{h}_{qc}", tag="recr")
                nc.vector.reciprocal(out=rec_r, in_=den_r)
                rec = smallp.tile([1, QF], bf16, name=f"rec_{h}_{qc}", tag="rec")
                nc.gpsimd.dma_start(out=rec, in_=rec_r)
                rec_rows[(h, qc)] = rec

            def attn_v_B(h, qc):
                av = av_tiles[(h, qc)]
                bc = ps_mm.tile([DK, QF], f32, name=f"bc_{h}_{qc}", tag="mm")
                nc.tensor.matmul(
                    bc, lhsT=ones64, rhs=rec_rows[(h, qc)], start=True, stop=True
                )
                bcs = smallp.tile([DK, QF], f32, name=f"bcs_{h}_{qc}", tag="bcs")
                nc.scalar.copy(out=bcs, in_=bc)
                nc.vector.tensor_tensor(
                    out=aoT_sb[
                        (h % 2) * DK : (h % 2) * DK + DK,
                        h // 2,
                        qc * QF : (qc + 1) * QF,
                    ],
                    in0=av[0:DK, :],
                    in1=bcs,
                    op=MULT,
                )

            yp_tiles = {}

            def wo_partial(oc, qc):
                # head-dim chunks 0..2 (pairs 0-2) are final once attn_v(4,5)
                # lands; run 3 of 4 wo accumulation matmuls early and bank the
                # partial in bf16 SBUF so the tail is one matmul + add.
                ps = ps_mm.tile([P, QF], f32, name=f"ywp_{oc}_{qc}", tag="mm")
                for hdt in range(JC - 1):
                    nc.tensor.matmul(
                        ps,
                        lhsT=wo_sb[:, hdt, oc * P : (oc + 1) * P],
                        rhs=aoT_sb[:, hdt, qc * QF : (qc + 1) * QF],
                        start=(hdt == 0),
                        stop=(hdt == JC - 2),
                    )
                yp = outp.tile(
                    [P, QF], bf16, name=f"yp_{oc}_{qc}", tag="yp", bufs=16
                )
                nc.scalar.copy(out=yp, in_=ps)
                yp_tiles[(oc, qc)] = yp

            def wo_unit(oc, qc):
                ps = ps_mm.tile([P, QF], f32, name=f"y_{oc}_{qc}", tag="mm")
                nc.tensor.matmul(
                    ps,
                    lhsT=ident,
                    rhs=yp_tiles[(oc, qc)],
                    start=True,
                    stop=False,
                    skip_group_check=True,
                )
                nc.tensor.matmul(
                    ps,
                    lhsT=wo_sb[:, JC - 1, oc * P : (oc + 1) * P],
                    rhs=aoT_sb[:, JC - 1, qc * QF : (qc + 1) * QF],
                    start=False,
                    stop=True,
                    skip_group_check=True,
                )
                ysb = outp.tile([P, QF], f32, name=f"ysb_{oc}_{qc}", tag="y")
                nc.scalar.copy(out=ysb, in_=ps)
                nc.sync.dma_start(
                    out=yT_d[oc * P : (oc + 1) * P, qc * QF : (qc + 1) * QF],
                    in_=ysb,
                )

            def proj_qk_unit(jc, which, tch):
                nm, w_sb, x_sb, out_sb, scale = (
                    ("q", wq_sb, xq_sb, qT_sb, 0.125)
                    if which == 0
                    else ("k", wk_sb, xkv_sb, kT_sb, None)
                )
                ps = ps_mm.tile([P, QF], f32, name=f"pj{nm}_{jc}_{tch}", tag="mm")
                for ft in range(FT):
                    nc.tensor.matmul(
                        ps,
                        lhsT=w_sb[:, ft, jc * P : (jc + 1) * P],
                        rhs=x_sb[:, ft, tch * QF : (tch + 1) * QF],
                        start=(ft == 0),
                        stop=(ft == FT - 1),
                    )
                dst = out_sb[:, jc, tch * QF : (tch + 1) * QF]
                if scale is None:
                    nc.vector.tensor_copy(out=dst, in_=ps)
                else:
                    nc.vector.tensor_scalar_mul(dst, ps, scale)

            def proj_v_unit(tt):
                ps = ps_mm.tile([P, QF], f32, name=f"pjv_{tt}", tag="mm")
                for ft in range(FT):
                    nc.tensor.matmul(
                        ps,
                        lhsT=xkv_sb[:, ft, tt * P : (tt + 1) * P],
                        rhs=wv_sb[:, ft, :],
                        start=(ft == 0),
                        stop=(ft == FT - 1),
                    )
                nc.vector.tensor_copy(
                    out=v_sb[:, tt, :, 0:DK],
                    in_=ps.rearrange("p (h d) -> p h d", h=NH_L),
                )

            # ---- interleaved emission schedule ----
            # scores steps are ACT-paced (~2.3us exp per step vs ~1.3us of
            # PE matmul); fillers keep the PE queue stocked so it never
            # idles past the ~3.4us HAM window and stays at 2.4 GHz.
            def av_pair_fillers(h0, h1):
                # A then B with one unit of slack so the bc matmul (B) never
                # heads the PE queue before its reciprocal chain finishes.
                return [
                    lambda h=h0: attn_v_A(h, 0),
                    lambda h=h0: attn_v_A(h, 1),
                    lambda h=h0: attn_v_B(h, 0),
                    lambda h=h1: attn_v_A(h, 0),
                    lambda h=h0: attn_v_B(h, 1),
                    lambda h=h1: attn_v_A(h, 1),
                    lambda h=h1: attn_v_B(h, 0),
                    lambda h=h1: attn_v_B(h, 1),
                ]

            def qk_fillers(jc):
                return [
                    lambda w=w, t=t, jc=jc: proj_qk_unit(jc, w, t)
                    for w in range(2)
                    for t in range(QC)
                ]

            for which in range(2):
                for tch in range(QC):
                    proj_qk_unit(0, which, tch)

            fillers_by_pair = [
                [lambda tt=tt: proj_v_unit(tt) for tt in range(TT)] + qk_fillers(1),
                av_pair_fillers(0, 1) + qk_fillers(2),
                av_pair_fillers(2, 3) + qk_fillers(3),
                av_pair_fillers(4, 5),
            ]
            for hp in range(4):
                fillers = fillers_by_pair[hp]
                k = 0
                for mt in range(TT):
                    scores_step(hp, mt)
                    # spread fillers evenly across the 8 steps
                    want = (mt + 1) * len(fillers) // TT
                    while k < want:
                        fillers[k]()
                        k += 1
            wpA = [lambda oc=oc: wo_partial(oc, 0) for oc in range(FT)]
            wpB = [lambda oc=oc: wo_partial(oc, 1) for oc in range(FT)]
            tail = [
                lambda: attn_v_A(6, 0), wpA[0], wpA[1],
                lambda: attn_v_A(6, 1), wpA[2], wpA[3],
                lambda: attn_v_B(6, 0),
                lambda: attn_v_A(7, 0), wpA[4], wpA[5],
                lambda: attn_v_B(6, 1),
                lambda: attn_v_A(7, 1), wpA[6], wpA[7],
                lambda: attn_v_B(7, 0), wpB[0], wpB[1],
            ]
            tail += [lambda oc=oc: wo_unit(oc, 0) for oc in range(FT)]
            tail += [lambda: attn_v_B(7, 1), wpB[2], wpB[3]]
            tail += [w for w in wpB[4:]]
            tail += [lambda oc=oc: wo_unit(oc, 1) for oc in range(FT)]
            for f in tail:
                f()

    _split_waits(nc)
    _prog = nc
    return nc
def _in_maps(x_q, x_kv, bias, Wq, Wk, Wv, Wo):
    import ml_dtypes

    bf16 = ml_dtypes.bfloat16

    def cvt(a):
        return np.ascontiguousarray(a).astype(bf16)

    maps = []
    for c in range(N_CORES):
        b, g = c // 2, c % 2
        hd = slice(g * JL, (g + 1) * JL)
        hs = slice(g * NH_L, (g + 1) * NH_L)
        maps.append(
            {
                "xqT": cvt(x_q[b].T),
                "xkvT": cvt(x_kv[b].T),
                "wqT": cvt(Wq[hd, :].T),
                "wkT": cvt(Wk[hd, :].T),
                "wvT": cvt(Wv[hd, :].T),
                "woT": cvt(Wo[:, hd].T),
                "biasT": cvt(bias[b, hs].swapaxes(1, 2)),
            }
        )
    return maps


def _postprocess(results, x_q):
    y = np.empty((B, S, H), np.float32)
    for b in range(B):
        acc = results[2 * b]["yT"].astype(np.float32) + results[2 * b + 1][
            "yT"
        ].astype(np.float32)
        y[b] = x_q[b].astype(np.float32) + acc.T
    return y


def kernel(x_q, x_kv, bias, Wq, Wk, Wv, Wo):
    x_q = np.asarray(x_q)
    nc = _build()
    maps = _in_maps(x_q, np.asarray(x_kv), np.asarray(bias), np.asarray(Wq),
                    np.asarray(Wk), np.asarray(Wv), np.asarray(Wo))
    from concourse.bass_utils import run_bass_kernel_spmd

    res = run_bass_kernel_spmd(nc, maps, list(range(N_CORES)))
    return _postprocess(res.results, x_q)


# revision 27
# speedup vs baseline: 1.0288x; 1.0288x over previous
"""Trainium2 Bass kernel: transformer block (biased attention + residual).

Reference math (B=4, S=1024, H=1024, NH=16, DK=64):
    q = x_q @ Wq.T ; k = x_kv @ Wk.T ; v = x_kv @ Wv.T   (per-head reshape)
    scores = q k^T / sqrt(DK) + bias ; attn = softmax(scores)
    out = x_q + (attn v reshaped) @ Wo.T

Sharding: 8 cores = 4 batches x 2 head-groups (8 heads each). Each core
computes its (batch, head-group) slice; the host sums the two head-group
partial outputs per batch and adds the residual.

Per-core dataflow (all matmul inputs bf16, PSUM accumulation fp32):
    qT/kT = W_g x^T           (head_dim on partitions, seq on free)
    v     = x_kv @ Wv_g.T     (seq on partitions), padded with a ones column
    scoresT[k,q] = k_h q_h^T (pair-packed K=64 row tiles)
                   + I^T biasT (bias added via identity matmul into PSUM)
    expT  = exp(scoresT)      (no max subtraction needed: |scores| <~ 8)
    avT   = v_aug^T expT      -> rows 0..63 = attn out^T, row 64 = denom
    aoT   = avT[0:64] * broadcast(1/denom)   (K=1 matmul broadcast)
    yT    = Wo_g^T-contraction of aoT        (partial, fp32 out)
"""

import sys

import numpy as np

for _p in ("/opt/trn_rl_repo",):
    if _p not in sys.path:
        sys.path.append(_p)

B, S, H, NH = 4, 1024, 1024, 16
DK = 64
P = 128
NH_L = 8            # heads per core
JL = NH_L * DK      # 512 local head dims per core
FT = H // P         # 8 contraction tiles for projections
TT = S // P         # 8 seq tiles
JC = JL // P        # 4 local head-dim chunks of 128
QF = 512            # matmul moving free dim (one PSUM bank of fp32)
QC = S // QF        # 2 q chunks
N_CORES = 8


def _split_waits(nc, max_waits=1):
    """This walrus build rejects instructions carrying more than ~1 sem
    wait ("Too many sync wait commands" in setupSyncWait). Hoist surplus
    waits onto same-engine NoOps spliced immediately before the carrying
    instruction — same engine position, so semantics are unchanged."""
    import bass_rust
    import concourse.mybir as mybir

    n = 0
    for f in nc.m.functions:
        for bb in f.blocks:
            new_insts = []
            for inst in bb.instructions:
                si = inst.sync_info
                waits = list(si.on_wait) if si and si.on_wait else []
                if len(waits) > max_waits:
                    keep = waits[:max_waits]
                    extra = waits[max_waits:]
                    for i in range(0, len(extra), max_waits):
                        nop = mybir.InstNoOp(name=f"WSPLIT-{n}", ins=[], outs=[])
                        n += 1
                        nop.engine = inst.engine
                        nop.bass_nofuse = False
                        nop.debug = inst.debug
                        nop.sync_info = bass_rust.SyncInfo(
                            on_wait=extra[i : i + max_waits], on_update=[]
                        )
                        new_insts.append(nop)
                    si.on_wait = keep
                    inst.sync_info = si
                new_insts.append(inst)
            bb.instructions[:] = new_insts


_prog = None


def _build():
    global _prog
    if _prog is not None:
        return _prog

    import concourse.bass as bass
    import concourse.mybir as mybir
    import concourse.tile as tile
    from concourse.masks import make_identity

    f32 = mybir.dt.float32
    bf16 = mybir.dt.bfloat16
    EXP = mybir.ActivationFunctionType.Exp
    MULT = mybir.AluOpType.mult

    nc = bass.Bass()
    xqT_d = nc.declare_dram_parameter("xqT", [H, S], bf16, isOutput=False)
    xkvT_d = nc.declare_dram_parameter("xkvT", [H, S], bf16, isOutput=False)
    wqT_d = nc.declare_dram_parameter("wqT", [H, JL], bf16, isOutput=False)
    wkT_d = nc.declare_dram_parameter("wkT", [H, JL], bf16, isOutput=False)
    wvT_d = nc.declare_dram_parameter("wvT", [H, JL], bf16, isOutput=False)
    woT_d = nc.declare_dram_parameter("woT", [JL, H], bf16, isOutput=False)
    biasT_d = nc.declare_dram_parameter("biasT", [NH_L, S, S], bf16, isOutput=False)
    yT_d = nc.declare_dram_parameter("yT", [H, S], f32, isOutput=True)

    with tile.TileContext(nc) as tc:
        with (
            tc.tile_pool(name="singles", bufs=1) as singles,
            tc.tile_pool(name="biasp", bufs=3) as biasp,
            tc.tile_pool(name="expp", bufs=18) as expp,
            tc.tile_pool(name="smallp", bufs=3) as smallp,
            tc.tile_pool(name="outp", bufs=3) as outp,
            tc.tile_pool(name="ps_s", bufs=4, space="PSUM") as ps_s,
            tc.tile_pool(name="ps_mm", bufs=2, space="PSUM") as ps_mm,
            tc.tile_pool(name="ps_av", bufs=2, space="PSUM") as ps_av,
        ):
            xq_sb = singles.tile([P, FT, S], bf16)
            xkv_sb = singles.tile([P, FT, S], bf16)
            wq_sb = singles.tile([P, FT, JL], bf16)
            wk_sb = singles.tile([P, FT, JL], bf16)
            wv_sb = singles.tile([P, FT, JL], bf16)
            wo_sb = singles.tile([P, JC, H], bf16)
            qT_sb = singles.tile([P, JC, S], bf16)
            kT_sb = singles.tile([P, JC, S], bf16)
            v_sb = singles.tile([P, TT, NH_L, DK + 1], bf16)
            aoT_sb = singles.tile([P, JC, S], bf16)
            ident = singles.tile([P, P], bf16)
            ones64 = singles.tile([1, DK], bf16)

            make_identity(nc, ident)
            nc.vector.memset(ones64, 1.0)
            nc.vector.memset(v_sb[:, :, :, DK : DK + 1], 1.0)

            def load2(sb, dr, cols=None):
                drr = dr.rearrange("(n p) j -> p n j", p=P)
                for f2 in range(FT // 2):
                    s = slice(2 * f2, 2 * f2 + 2)
                    if cols is None:
                        nc.sync.dma_start(out=sb[:, s, :], in_=drr[:, s, :])
                    else:
                        nc.sync.dma_start(
                            out=sb[:, s, cols], in_=drr[:, s, cols]
                        )

            load2(wq_sb, wqT_d)
            load2(xq_sb, xqT_d, cols=slice(0, QF))
            load2(wk_sb, wkT_d)
            load2(xkv_sb, xkvT_d, cols=slice(0, QF))
            load2(xq_sb, xqT_d, cols=slice(QF, S))
            load2(xkv_sb, xkvT_d, cols=slice(QF, S))
            for hdt in range(JC):
                nc.sync.dma_start(
                    out=wo_sb[:, hdt, :], in_=woT_d[hdt * P : (hdt + 1) * P, :]
                )

            bias_pref = {}

            def bias_fetch(hp, mt):
                out = []
                for i in range(2):
                    h = 2 * hp + i
                    bt = biasp.tile(
                        [P, S], bf16, name=f"bias_{h}_{mt}", tag=f"bias{i}"
                    )
                    nc.sync.dma_start(
                        out=bt, in_=biasT_d[h, mt * P : (mt + 1) * P, :]
                    )
                    out.append(bt)
                return out

            for _mt in range(3):
                bias_pref[(0, _mt)] = bias_fetch(0, _mt)
            load2(wv_sb, wvT_d)

            # (HAM warm-up) ~30 back-to-back tiny matmuls while the input
            # DMAs land, so the PE clock is at 8/8 when real work starts.
            warm_ps = ps_mm.tile([P, P], f32, name="warm", tag="mm")
            for _ in range(90):
                nc.tensor.matmul(warm_ps, lhsT=ident, rhs=ident,
                                 start=True, stop=True, skip_group_check=True)

            exp_tiles = {}

            def proj_qk(jc):
                for nm, w_sb, x_sb, out_sb, scale in (
                    ("q", wq_sb, xq_sb, qT_sb, 0.125),
                    ("k", wk_sb, xkv_sb, kT_sb, None),
                ):
                    for tch in range(QC):
                        ps = ps_mm.tile(
                            [P, QF], f32, name=f"pj{nm}_{jc}_{tch}", tag="mm"
                        )
                        for ft in range(FT):
                            nc.tensor.matmul(
                                ps,
                                lhsT=w_sb[:, ft, jc * P : (jc + 1) * P],
                                rhs=x_sb[:, ft, tch * QF : (tch + 1) * QF],
                                start=(ft == 0),
                                stop=(ft == FT - 1),
                            )
                        dst = out_sb[:, jc, tch * QF : (tch + 1) * QF]
                        if scale is None:
                            nc.vector.tensor_copy(out=dst, in_=ps)
                        else:
                            nc.vector.tensor_scalar_mul(dst, ps, scale)

            def proj_v():
                for tt in range(TT):
                    ps = ps_mm.tile([P, QF], f32, name=f"pjv_{tt}", tag="mm")
                    for ft in range(FT):
                        nc.tensor.matmul(
                            ps,
                            lhsT=xkv_sb[:, ft, tt * P : (tt + 1) * P],
                            rhs=wv_sb[:, ft, :],
                            start=(ft == 0),
                            stop=(ft == FT - 1),
                        )
                    nc.vector.tensor_copy(
                        out=v_sb[:, tt, :, 0:DK],
                        in_=ps.rearrange("p (h d) -> p h d", h=NH_L),
                    )

            def scores_step(hp, mt):
                if (hp, mt) in bias_pref:
                    btiles = bias_pref.pop((hp, mt))
                else:
                    btiles = bias_fetch(hp, mt)
                ptiles = [[None] * QC for _ in range(2)]
                for qc in range(QC):
                    for i in range(2):
                        # bias matmul first: it depends only on the bias DMA,
                        # so it can run before qT/kT chunks are projected
                        ps = ps_s.tile(
                            [P, QF], f32, name=f"sc_{hp}_{mt}_{i}_{qc}", tag="sc"
                        )
                        nc.tensor.matmul(
                            ps,
                            lhsT=ident,
                            rhs=btiles[i][:, qc * QF : (qc + 1) * QF],
                            start=True,
                            stop=False,
                            skip_group_check=True,
                        )
                        ptiles[i][qc] = ps
                    for i in range(2):
                        jr = i * DK
                        nc.tensor.matmul(
                            ptiles[i][qc],
                            lhsT=kT_sb[jr : jr + DK, hp, mt * P : (mt + 1) * P],
                            rhs=qT_sb[jr : jr + DK, hp, qc * QF : (qc + 1) * QF],
                            start=False,
                            stop=True,
                            skip_group_check=True,
                        )
                for i in range(2):
                    h = 2 * hp + i
                    et = expp.tile([P, S], bf16, name=f"exp_{h}_{mt}", tag="exp")
                    exp_tiles[(h, mt)] = et
                    for qc in range(QC):
                        nc.scalar.activation(
                            out=et[:, qc * QF : (qc + 1) * QF],
                            in_=ptiles[i][qc],
                            func=EXP,
                        )

            av_tiles = {}
            rec_rows = {}

            def attn_v_A(h, qc):
                # attn@v accumulation + cheap reciprocal of the denominator
                # row (reshaped (1,512)->(128,4) by DMA so the DVE recip runs
                # at FD=4 instead of FD=512; back-DMA on gpsimd casts bf16).
                av = ps_av.tile([P, QF], f32, name=f"av_{h}_{qc}", tag="av")
                av_tiles[(h, qc)] = av
                for mt in range(TT):
                    nc.tensor.matmul(
                        av[0 : DK + 1, :],
                        lhsT=v_sb[:, mt, h, :],
                        rhs=exp_tiles[(h, mt)][:, qc * QF : (qc + 1) * QF],
                        start=(mt == 0),
                        stop=(mt == TT - 1),
                    )
                den = smallp.tile([1, QF], f32, name=f"den_{h}_{qc}", tag="den")
                nc.vector.tensor_copy(out=den, in_=av[DK : DK + 1, :])
                den_r = smallp.tile([P, QF // P], f32, name=f"denr_{h}_{qc}", tag="denr")
                nc.sync.dma_start(out=den_r, in_=den)
                rec_r = smallp.tile([P, QF // P], f32, name=f"recr_{h}_{qc}", tag="recr")
                nc.vector.reciprocal(out=rec_r, in_=den_r)
                rec = smallp.tile([1, QF], bf16, name=f"rec_{h}_{qc}", tag="rec")
                nc.gpsimd.dma_start(out=rec, in_=rec_r)
                rec_rows[(h, qc)] = rec

            def attn_v_B(h, qc):
                av = av_tiles[(h, qc)]
                bc = ps_mm.tile([DK, QF], f32, name=f"bc_{h}_{qc}", tag="mm")
                nc.tensor.matmul(
                    bc, lhsT=ones64, rhs=rec_rows[(h, qc)], start=True, stop=True
                )
                bcs = smallp.tile([DK, QF], f32, name=f"bcs_{h}_{qc}", tag="bcs")
                nc.scalar.copy(out=bcs, in_=bc)
                nc.vector.tensor_tensor(
                    out=aoT_sb[
                        (h % 2) * DK : (h % 2) * DK + DK,
                        h // 2,
                        qc * QF : (qc + 1) * QF,
                    ],
                    in0=av[0:DK, :],
                    in1=bcs,
                    op=MULT,
                )

            yp_tiles = {}

            def wo_partial(oc, qc):
                # head-dim chunks 0..2 (pairs 0-2) are final once attn_v(4,5)
                # lands; run 3 of 4 wo accumulation matmuls early and bank the
                # partial in bf16 SBUF so the tail is one matmul + add.
                ps = ps_mm.tile([P, QF], f32, name=f"ywp_{oc}_{qc}", tag="mm")
                for hdt in range(JC - 1):
                    nc.tensor.matmul(
                        ps,
                        lhsT=wo_sb[:, hdt, oc * P : (oc + 1) * P],
                        rhs=aoT_sb[:, hdt, qc * QF : (qc + 1) * QF],
                        start=(hdt == 0),
                        stop=(hdt == JC - 2),
                    )
                yp = outp.tile(
                    [P, QF], bf16, name=f"yp_{oc}_{qc}", tag="yp", bufs=16
                )
                nc.scalar.copy(out=yp, in_=ps)
                yp_tiles[(oc, qc)] = yp

            def wo_unit(oc, qc):
                ps = ps_mm.tile([P, QF], f32, name=f"y_{oc}_{qc}", tag="mm")
                nc.tensor.matmul(
                    ps,
                    lhsT=ident,
                    rhs=yp_tiles[(oc, qc)],
                    start=True,
                    stop=False,
                    skip_group_check=True,
                )
                nc.tensor.matmul(
                    ps,
                    lhsT=wo_sb[:, JC - 1, oc * P : (oc + 1) * P],
                    rhs=aoT_sb[:, JC - 1, qc * QF : (qc + 1) * QF],
                    start=False,
                    stop=True,
                    skip_group_check=True,
                )
                ysb = outp.tile([P, QF], f32, name=f"ysb_{oc}_{qc}", tag="y")
                nc.scalar.copy(out=ysb, in_=ps)
                nc.sync.dma_start(
                    out=yT_d[oc * P : (oc + 1) * P, qc * QF : (qc + 1) * QF],
                    in_=ysb,
                )

            def proj_qk_unit(jc, which, tch):
                nm, w_sb, x_sb, out_sb, scale = (
                    ("q", wq_sb, xq_sb, qT_sb, 0.125)
                    if which == 0
                    else ("k", wk_sb, xkv_sb, kT_sb, None)
                )
                ps = ps_mm.tile([P, QF], f32, name=f"pj{nm}_{jc}_{tch}", tag="mm")
                for ft in range(FT):
                    nc.tensor.matmul(
                        ps,
                        lhsT=w_sb[:, ft, jc * P : (jc + 1) * P],
                        rhs=x_sb[:, ft, tch * QF : (tch + 1) * QF],
                        start=(ft == 0),
                        stop=(ft == FT - 1),
                    )
                dst = out_sb[:, jc, tch * QF : (tch + 1) * QF]
                if scale is None:
                    nc.vector.tensor_copy(out=dst, in_=ps)
                else:
                    nc.vector.tensor_scalar_mul(dst, ps, scale)

            def proj_v_unit(tt):
                ps = ps_mm.tile([P, QF], f32, name=f"pjv_{tt}", tag="mm")
                for ft in range(FT):
                    nc.tensor.matmul(
                        ps,
                        lhsT=xkv_sb[:, ft, tt * P : (tt + 1) * P],
                        rhs=wv_sb[:, ft, :],
                        start=(ft == 0),
                        stop=(ft == FT - 1),
                    )
                nc.vector.tensor_copy(
                    out=v_sb[:, tt, :, 0:DK],
                    in_=ps.rearrange("p (h d) -> p h d", h=NH_L),
                )

            # ---- interleaved emission schedule ----
            # scores steps are ACT-paced (~2.3us exp per step vs ~1.3us of
            # PE matmul); fillers keep the PE queue stocked so it never
            # idles past the ~3.4us HAM window and stays at 2.4 GHz.
            def av_pair_fillers(h0, h1):
                # A then B with one unit of slack so the bc matmul (B) never
                # heads the PE queue before its reciprocal chain finishes.
                return [
                    lambda h=h0: attn_v_A(h, 0),
                    lambda h=h0: attn_v_A(h, 1),
                    lambda h=h0: attn_v_B(h, 0),
                    lambda h=h1: attn_v_A(h, 0),
                    lambda h=h0: attn_v_B(h, 1),
                    lambda h=h1: attn_v_A(h, 1),
                    lambda h=h1: attn_v_B(h, 0),
                    lambda h=h1: attn_v_B(h, 1),
                ]

            def qk_fillers(jc):
                return [
                    lambda w=w, t=t, jc=jc: proj_qk_unit(jc, w, t)
                    for w in range(2)
                    for t in range(QC)
                ]

            for which in range(2):
                for tch in range(QC):
                    proj_qk_unit(0, which, tch)

            fillers_by_pair = [
                [lambda tt=tt: proj_v_unit(tt) for tt in range(TT)] + qk_fillers(1),
                av_pair_fillers(0, 1) + qk_fillers(2),
                av_pair_fillers(2, 3) + qk_fillers(3),
                av_pair_fillers(4, 5),
            ]
            for hp in range(4):
                fillers = fillers_by_pair[hp]
                k = 0
                for mt in range(TT):
                    scores_step(hp, mt)
                    # spread fillers evenly across the 8 steps
                    want = (mt + 1) * len(fillers) // TT
                    while k < want:
                        fillers[k]()
                        k += 1
            wp = [
                lambda oc=oc, qc=qc: wo_partial(oc, qc)
                for oc in range(FT)
                for qc in range(QC)
            ]
            tail = [
                lambda: attn_v_A(6, 0), wp[0], wp[1],
                lambda: attn_v_A(6, 1), wp[2], wp[3],
                lambda: attn_v_B(6, 0),
                lambda: attn_v_A(7, 0), wp[4], wp[5],
                lambda: attn_v_B(6, 1),
                lambda: attn_v_A(7, 1), wp[6], wp[7],
                lambda: attn_v_B(7, 0), wp[8], wp[9], wp[10], wp[11],
                lambda: attn_v_B(7, 1), wp[12], wp[13], wp[14], wp[15],
            ]
            for f in tail:
                f()
            for oc in range(FT):
                for qc in range(QC):
                    wo_unit(oc, qc)

    _split_waits(nc)
    _prog = nc
    return nc
def _in_maps(x_q, x_kv, bias, Wq, Wk, Wv, Wo):
    import ml_dtypes

    bf16 = ml_dtypes.bfloat16

    def cvt(a):
        return np.ascontiguousarray(a).astype(bf16)

    maps = []
    for c in range(N_CORES):
        b, g = c // 2, c % 2
        hd = slice(g * JL, (g + 1) * JL)
        hs = slice(g * NH_L, (g + 1) * NH_L)
        maps.append(
            {
                "xqT": cvt(x_q[b].T),
                "xkvT": cvt(x_kv[b].T),
                "wqT": cvt(Wq[hd, :].T),
                "wkT": cvt(Wk[hd, :].T),
                "wvT": cvt(Wv[hd, :].T),
                "woT": cvt(Wo[:, hd].T),
                "biasT": cvt(bias[b, hs].swapaxes(1, 2)),
            }
        )
    return maps


def _postprocess(results, x_q):
    y = np.empty((B, S, H), np.float32)
    for b in range(B):
        acc = results[2 * b]["yT"].astype(np.float32) + results[2 * b + 1][
            "yT"
        ].astype(np.float32)
        y[b] = x_q[b].astype(np.float32) + acc.T
    return y


def kernel(x_q, x_kv, bias, Wq, Wk, Wv, Wo):
    x_q = np.asarray(x_q)
    nc = _build()
    maps = _in_maps(x_q, np.asarray(x_kv), np.asarray(bias), np.asarray(Wq),
                    np.asarray(Wk), np.asarray(Wv), np.asarray(Wo))
    from concourse.bass_utils import run_bass_kernel_spmd

    res = run_bass_kernel_spmd(nc, maps, list(range(N_CORES)))
    return _postprocess(res.results, x_q)


# revision 29
# speedup vs baseline: 1.0590x; 1.0294x over previous
"""Trainium2 Bass kernel: transformer block (biased attention + residual).

Reference math (B=4, S=1024, H=1024, NH=16, DK=64):
    q = x_q @ Wq.T ; k = x_kv @ Wk.T ; v = x_kv @ Wv.T   (per-head reshape)
    scores = q k^T / sqrt(DK) + bias ; attn = softmax(scores)
    out = x_q + (attn v reshaped) @ Wo.T

Sharding: 8 cores = 4 batches x 2 head-groups (8 heads each). Each core
computes its (batch, head-group) slice; the host sums the two head-group
partial outputs per batch and adds the residual.

Per-core dataflow (all matmul inputs bf16, PSUM accumulation fp32):
    qT/kT = W_g x^T           (head_dim on partitions, seq on free)
    v     = x_kv @ Wv_g.T     (seq on partitions), padded with a ones column
    scoresT[k,q] = k_h q_h^T (pair-packed K=64 row tiles)
                   + I^T biasT (bias added via identity matmul into PSUM)
    expT  = exp(scoresT)      (no max subtraction needed: |scores| <~ 8)
    avT   = v_aug^T expT      -> rows 0..63 = attn out^T, row 64 = denom
    aoT   = avT[0:64] * broadcast(1/denom)   (K=1 matmul broadcast)
    yT    = Wo_g^T-contraction of aoT        (partial, fp32 out)
"""

import sys

import numpy as np

for _p in ("/opt/trn_rl_repo",):
    if _p not in sys.path:
        sys.path.append(_p)

B, S, H, NH = 4, 1024, 1024, 16
DK = 64
P = 128
NH_L = 8            # heads per core
JL = NH_L * DK      # 512 local head dims per core
FT = H // P         # 8 contraction tiles for projections
TT = S // P         # 8 seq tiles
JC = JL // P        # 4 local head-dim chunks of 128
QF = 512            # matmul moving free dim (one PSUM bank of fp32)
QC = S // QF        # 2 q chunks
N_CORES = 8


def _split_waits(nc, max_waits=1):
    """This walrus build rejects instructions carrying more than ~1 sem
    wait ("Too many sync wait commands" in setupSyncWait). Hoist surplus
    waits onto same-engine NoOps spliced immediately before the carrying
    instruction — same engine position, so semantics are unchanged."""
    import bass_rust
    import concourse.mybir as mybir

    n = 0
    for f in nc.m.functions:
        for bb in f.blocks:
            new_insts = []
            for inst in bb.instructions:
                si = inst.sync_info
                waits = list(si.on_wait) if si and si.on_wait else []
                if len(waits) > max_waits:
                    keep = waits[:max_waits]
                    extra = waits[max_waits:]
                    for i in range(0, len(extra), max_waits):
                        nop = mybir.InstNoOp(name=f"WSPLIT-{n}", ins=[], outs=[])
                        n += 1
                        nop.engine = inst.engine
                        nop.bass_nofuse = False
                        nop.debug = inst.debug
                        nop.sync_info = bass_rust.SyncInfo(
                            on_wait=extra[i : i + max_waits], on_update=[]
                        )
                        new_insts.append(nop)
                    si.on_wait = keep
                    inst.sync_info = si
                new_insts.append(inst)
            bb.instructions[:] = new_insts


_prog = None


def _build():
    global _prog
    if _prog is not None:
        return _prog

    import concourse.bass as bass
    import concourse.mybir as mybir
    import concourse.tile as tile
    from concourse.masks import make_identity

    f32 = mybir.dt.float32
    bf16 = mybir.dt.bfloat16
    EXP = mybir.ActivationFunctionType.Exp
    MULT = mybir.AluOpType.mult

    nc = bass.Bass()
    xqT_d = nc.declare_dram_parameter("xqT", [H, S], bf16, isOutput=False)
    xkvT_d = nc.declare_dram_parameter("xkvT", [H, S], bf16, isOutput=False)
    wqT_d = nc.declare_dram_parameter("wqT", [H, JL], bf16, isOutput=False)
    wkT_d = nc.declare_dram_parameter("wkT", [H, JL], bf16, isOutput=False)
    wvT_d = nc.declare_dram_parameter("wvT", [H, JL], bf16, isOutput=False)
    woT_d = nc.declare_dram_parameter("woT", [JL, H], bf16, isOutput=False)
    biasT_d = nc.declare_dram_parameter("biasT", [NH_L, S, S], bf16, isOutput=False)
    yT_d = nc.declare_dram_parameter("yT", [H, S], f32, isOutput=True)

    with tile.TileContext(nc) as tc:
        with (
            tc.tile_pool(name="singles", bufs=1) as singles,
            tc.tile_pool(name="biasp", bufs=3) as biasp,
            tc.tile_pool(name="expp", bufs=18) as expp,
            tc.tile_pool(name="smallp", bufs=3) as smallp,
            tc.tile_pool(name="outp", bufs=3) as outp,
            tc.tile_pool(name="ps_s", bufs=4, space="PSUM") as ps_s,
            tc.tile_pool(name="ps_mm", bufs=2, space="PSUM") as ps_mm,
            tc.tile_pool(name="ps_av", bufs=2, space="PSUM") as ps_av,
        ):
            xq_sb = singles.tile([P, FT, S], bf16)
            xkv_sb = singles.tile([P, FT, S], bf16)
            wq_sb = singles.tile([P, FT, JL], bf16)
            wk_sb = singles.tile([P, FT, JL], bf16)
            wv_sb = singles.tile([P, FT, JL], bf16)
            wo_sb = singles.tile([P, JC, H], bf16)
            qT_sb = singles.tile([P, JC, S], bf16)
            kT_sb = singles.tile([P, JC, S], bf16)
            v_sb = singles.tile([P, TT, NH_L, DK + 1], bf16)
            aoT_sb = singles.tile([P, JC, S], bf16)
            ident = singles.tile([P, P], bf16)
            ones64 = singles.tile([1, DK], bf16)

            make_identity(nc, ident)
            nc.vector.memset(ones64, 1.0)
            nc.vector.memset(v_sb[:, :, :, DK : DK + 1], 1.0)

            def load2(sb, dr, cols=None):
                drr = dr.rearrange("(n p) j -> p n j", p=P)
                for f2 in range(FT // 2):
                    s = slice(2 * f2, 2 * f2 + 2)
                    if cols is None:
                        nc.sync.dma_start(out=sb[:, s, :], in_=drr[:, s, :])
                    else:
                        nc.sync.dma_start(
                            out=sb[:, s, cols], in_=drr[:, s, cols]
                        )

            load2(wq_sb, wqT_d)
            load2(xq_sb, xqT_d, cols=slice(0, QF))
            load2(wk_sb, wkT_d)
            load2(xkv_sb, xkvT_d, cols=slice(0, QF))
            load2(xq_sb, xqT_d, cols=slice(QF, S))
            load2(xkv_sb, xkvT_d, cols=slice(QF, S))

            bias_pref = {}

            def bias_fetch(hp, mt):
                out = []
                for i in range(2):
                    h = 2 * hp + i
                    bt = biasp.tile(
                        [P, S], bf16, name=f"bias_{h}_{mt}", tag=f"bias{i}"
                    )
                    nc.sync.dma_start(
                        out=bt, in_=biasT_d[h, mt * P : (mt + 1) * P, :]
                    )
                    out.append(bt)
                return out

            for _mt in range(3):
                bias_pref[(0, _mt)] = bias_fetch(0, _mt)
            load2(wv_sb, wvT_d)
            for hdt in range(JC):
                nc.sync.dma_start(
                    out=wo_sb[:, hdt, :], in_=woT_d[hdt * P : (hdt + 1) * P, :]
                )

            # (HAM warm-up) ~30 back-to-back tiny matmuls while the input
            # DMAs land, so the PE clock is at 8/8 when real work starts.
            warm_ps = ps_mm.tile([P, P], f32, name="warm", tag="mm")
            for _ in range(24):
                nc.tensor.matmul(warm_ps, lhsT=ident, rhs=ident,
                                 start=True, stop=True, skip_group_check=True)
            warm2_ps = ps_av.tile([P, P], f32, name="warm2", tag="av")
            for _ in range(56):
                nc.tensor.matmul(warm2_ps, lhsT=ident, rhs=ident,
                                 start=True, stop=True, skip_group_check=True)

            exp_tiles = {}

            def proj_qk(jc):
                for nm, w_sb, x_sb, out_sb, scale in (
                    ("q", wq_sb, xq_sb, qT_sb, 0.125),
                    ("k", wk_sb, xkv_sb, kT_sb, None),
                ):
                    for tch in range(QC):
                        ps = ps_mm.tile(
                            [P, QF], f32, name=f"pj{nm}_{jc}_{tch}", tag="mm"
                        )
                        for ft in range(FT):
                            nc.tensor.matmul(
                                ps,
                                lhsT=w_sb[:, ft, jc * P : (jc + 1) * P],
                                rhs=x_sb[:, ft, tch * QF : (tch + 1) * QF],
                                start=(ft == 0),
                                stop=(ft == FT - 1),
                            )
                        dst = out_sb[:, jc, tch * QF : (tch + 1) * QF]
                        if scale is None:
                            nc.vector.tensor_copy(out=dst, in_=ps)
                        else:
                            nc.vector.tensor_scalar_mul(dst, ps, scale)

            def proj_v():
                for tt in range(TT):
                    ps = ps_mm.tile([P, QF], f32, name=f"pjv_{tt}", tag="mm")
                    for ft in range(FT):
                        nc.tensor.matmul(
                            ps,
                            lhsT=xkv_sb[:, ft, tt * P : (tt + 1) * P],
                            rhs=wv_sb[:, ft, :],
                            start=(ft == 0),
                            stop=(ft == FT - 1),
                        )
                    nc.vector.tensor_copy(
                        out=v_sb[:, tt, :, 0:DK],
                        in_=ps.rearrange("p (h d) -> p h d", h=NH_L),
                    )

            def scores_step(hp, mt):
                if (hp, mt) in bias_pref:
                    btiles = bias_pref.pop((hp, mt))
                else:
                    btiles = bias_fetch(hp, mt)
                ptiles = [[None] * QC for _ in range(2)]
                for qc in range(QC):
                    for i in range(2):
                        # bias matmul first: it depends only on the bias DMA,
                        # so it can run before qT/kT chunks are projected
                        ps = ps_s.tile(
                            [P, QF], f32, name=f"sc_{hp}_{mt}_{i}_{qc}", tag="sc"
                        )
                        nc.tensor.matmul(
                            ps,
                            lhsT=ident,
                            rhs=btiles[i][:, qc * QF : (qc + 1) * QF],
                            start=True,
                            stop=False,
                            skip_group_check=True,
                        )
                        ptiles[i][qc] = ps
                    for i in range(2):
                        jr = i * DK
                        nc.tensor.matmul(
                            ptiles[i][qc],
                            lhsT=kT_sb[jr : jr + DK, hp, mt * P : (mt + 1) * P],
                            rhs=qT_sb[jr : jr + DK, hp, qc * QF : (qc + 1) * QF],
                            start=False,
                            stop=True,
                            skip_group_check=True,
                        )
                for i in range(2):
                    h = 2 * hp + i
                    et = expp.tile([P, S], bf16, name=f"exp_{h}_{mt}", tag="exp")
                    exp_tiles[(h, mt)] = et
                    for qc in range(QC):
                        nc.scalar.activation(
                            out=et[:, qc * QF : (qc + 1) * QF],
                            in_=ptiles[i][qc],
                            func=EXP,
                        )

            av_tiles = {}
            rec_rows = {}

            def attn_v_A(h, qc):
                # attn@v accumulation + cheap reciprocal of the denominator
                # row (reshaped (1,512)->(128,4) by DMA so the DVE recip runs
                # at FD=4 instead of FD=512; back-DMA on gpsimd casts bf16).
                av = ps_av.tile([P, QF], f32, name=f"av_{h}_{qc}", tag="av")
                av_tiles[(h, qc)] = av
                for mt in range(TT):
                    nc.tensor.matmul(
                        av[0 : DK + 1, :],
                        lhsT=v_sb[:, mt, h, :],
                        rhs=exp_tiles[(h, mt)][:, qc * QF : (qc + 1) * QF],
                        start=(mt == 0),
                        stop=(mt == TT - 1),
                    )
                den = smallp.tile([1, QF], f32, name=f"den_{h}_{qc}", tag="den")
                nc.vector.tensor_copy(out=den, in_=av[DK : DK + 1, :])
                den_r = smallp.tile([P, QF // P], f32, name=f"denr_{h}_{qc}", tag="denr")
                nc.sync.dma_start(out=den_r, in_=den)
                rec_r = smallp.tile([P, QF // P], f32, name=f"recr_{h}_{qc}", tag="recr")
                nc.vector.reciprocal(out=rec_r, in_=den_r)
                rec = smallp.tile([1, QF], bf16, name=f"rec_{h}_{qc}", tag="rec")
                nc.gpsimd.dma_start(out=rec, in_=rec_r)
                rec_rows[(h, qc)] = rec

            def attn_v_B(h, qc):
                av = av_tiles[(h, qc)]
                bc = ps_mm.tile([DK, QF], f32, name=f"bc_{h}_{qc}", tag="mm")
                nc.tensor.matmul(
                    bc, lhsT=ones64, rhs=rec_rows[(h, qc)], start=True, stop=True
                )
                bcs = smallp.tile([DK, QF], f32, name=f"bcs_{h}_{qc}", tag="bcs")
                nc.scalar.copy(out=bcs, in_=bc)
                nc.vector.tensor_tensor(
                    out=aoT_sb[
                        (h % 2) * DK : (h % 2) * DK + DK,
                        h // 2,
                        qc * QF : (qc + 1) * QF,
                    ],
                    in0=av[0:DK, :],
                    in1=bcs,
                    op=MULT,
                )

            yp_tiles = {}

            def wo_partial(oc, qc):
                # head-dim chunks 0..2 (pairs 0-2) are final once attn_v(4,5)
                # lands; run 3 of 4 wo accumulation matmuls early and bank the
                # partial in bf16 SBUF so the tail is one matmul + add.
                ps = ps_mm.tile([P, QF], f32, name=f"ywp_{oc}_{qc}", tag="mm")
                for hdt in range(JC - 1):
                    nc.tensor.matmul(
                        ps,
                        lhsT=wo_sb[:, hdt, oc * P : (oc + 1) * P],
                        rhs=aoT_sb[:, hdt, qc * QF : (qc + 1) * QF],
                        start=(hdt == 0),
                        stop=(hdt == JC - 2),
                    )
                yp = outp.tile(
                    [P, QF], bf16, name=f"yp_{oc}_{qc}", tag="yp", bufs=16
                )
                nc.scalar.copy(out=yp, in_=ps)
                yp_tiles[(oc, qc)] = yp

            def wo_unit(oc, qc):
                ps = ps_mm.tile([P, QF], f32, name=f"y_{oc}_{qc}", tag="mm")
                nc.tensor.matmul(
                    ps,
                    lhsT=ident,
                    rhs=yp_tiles[(oc, qc)],
                    start=True,
                    stop=False,
                    skip_group_check=True,
                )
                nc.tensor.matmul(
                    ps,
                    lhsT=wo_sb[:, JC - 1, oc * P : (oc + 1) * P],
                    rhs=aoT_sb[:, JC - 1, qc * QF : (qc + 1) * QF],
                    start=False,
                    stop=True,
                    skip_group_check=True,
                )
                ysb = outp.tile([P, QF], f32, name=f"ysb_{oc}_{qc}", tag="y")
                nc.scalar.copy(out=ysb, in_=ps)
                nc.sync.dma_start(
                    out=yT_d[oc * P : (oc + 1) * P, qc * QF : (qc + 1) * QF],
                    in_=ysb,
                )

            def proj_qk_unit(jc, which, tch):
                nm, w_sb, x_sb, out_sb, scale = (
                    ("q", wq_sb, xq_sb, qT_sb, 0.125)
                    if which == 0
                    else ("k", wk_sb, xkv_sb, kT_sb, None)
                )
                ps = ps_mm.tile([P, QF], f32, name=f"pj{nm}_{jc}_{tch}", tag="mm")
                for ft in range(FT):
                    nc.tensor.matmul(
                        ps,
                        lhsT=w_sb[:, ft, jc * P : (jc + 1) * P],
                        rhs=x_sb[:, ft, tch * QF : (tch + 1) * QF],
                        start=(ft == 0),
                        stop=(ft == FT - 1),
                    )
                dst = out_sb[:, jc, tch * QF : (tch + 1) * QF]
                if scale is None:
                    nc.vector.tensor_copy(out=dst, in_=ps)
                else:
                    nc.vector.tensor_scalar_mul(dst, ps, scale)

            def proj_v_unit(tt):
                ps = ps_mm.tile([P, QF], f32, name=f"pjv_{tt}", tag="mm")
                for ft in range(FT):
                    nc.tensor.matmul(
                        ps,
                        lhsT=xkv_sb[:, ft, tt * P : (tt + 1) * P],
                        rhs=wv_sb[:, ft, :],
                        start=(ft == 0),
                        stop=(ft == FT - 1),
                    )
                nc.vector.tensor_copy(
                    out=v_sb[:, tt, :, 0:DK],
                    in_=ps.rearrange("p (h d) -> p h d", h=NH_L),
                )

            # ---- interleaved emission schedule ----
            # scores steps are ACT-paced (~2.3us exp per step vs ~1.3us of
            # PE matmul); fillers keep the PE queue stocked so it never
            # idles past the ~3.4us HAM window and stays at 2.4 GHz.
            def av_pair_fillers(h0, h1):
                # A then B with one unit of slack so the bc matmul (B) never
                # heads the PE queue before its reciprocal chain finishes.
                return [
                    lambda h=h0: attn_v_A(h, 0),
                    lambda h=h0: attn_v_A(h, 1),
                    lambda h=h0: attn_v_B(h, 0),
                    lambda h=h1: attn_v_A(h, 0),
                    lambda h=h0: attn_v_B(h, 1),
                    lambda h=h1: attn_v_A(h, 1),
                    lambda h=h1: attn_v_B(h, 0),
                    lambda h=h1: attn_v_B(h, 1),
                ]

            def qk_fillers(jc):
                return [
                    lambda w=w, t=t, jc=jc: proj_qk_unit(jc, w, t)
                    for w in range(2)
                    for t in range(QC)
                ]

            for which in range(2):
                for tch in range(QC):
                    proj_qk_unit(0, which, tch)

            fillers_by_pair = [
                [lambda tt=tt: proj_v_unit(tt) for tt in range(TT)] + qk_fillers(1),
                av_pair_fillers(0, 1) + qk_fillers(2),
                av_pair_fillers(2, 3) + qk_fillers(3),
                av_pair_fillers(4, 5),
            ]
            for hp in range(4):
                fillers = fillers_by_pair[hp]
                k = 0
                for mt in range(TT):
                    scores_step(hp, mt)
                    # spread fillers evenly across the 8 steps
                    want = (mt + 1) * len(fillers) // TT
                    while k < want:
                        fillers[k]()
                        k += 1
            wp = [
                lambda oc=oc, qc=qc: wo_partial(oc, qc)
                for oc in range(FT)
                for qc in range(QC)
            ]
            tail = [
                lambda: attn_v_A(6, 0), wp[0], wp[1],
                lambda: attn_v_A(6, 1), wp[2], wp[3],
                lambda: attn_v_B(6, 0),
                lambda: attn_v_A(7, 0), wp[4], wp[5],
                lambda: attn_v_B(6, 1),
                lambda: attn_v_A(7, 1), wp[6], wp[7],
                lambda: attn_v_B(7, 0), wp[8], wp[9], wp[10], wp[11],
                lambda: attn_v_B(7, 1), wp[12], wp[13], wp[14], wp[15],
            ]
            for f in tail:
                f()
            for oc in range(FT):
                for qc in range(QC):
                    wo_unit(oc, qc)

    _split_waits(nc)
    _prog = nc
    return nc
def _in_maps(x_q, x_kv, bias, Wq, Wk, Wv, Wo):
    import ml_dtypes

    bf16 = ml_dtypes.bfloat16

    def cvt(a):
        return np.ascontiguousarray(a).astype(bf16)

    maps = []
    for c in range(N_CORES):
        b, g = c // 2, c % 2
        hd = slice(g * JL, (g + 1) * JL)
        hs = slice(g * NH_L, (g + 1) * NH_L)
        maps.append(
            {
                "xqT": cvt(x_q[b].T),
                "xkvT": cvt(x_kv[b].T),
                "wqT": cvt(Wq[hd, :].T),
                "wkT": cvt(Wk[hd, :].T),
                "wvT": cvt(Wv[hd, :].T),
                "woT": cvt(Wo[:, hd].T),
                "biasT": cvt(bias[b, hs].swapaxes(1, 2)),
            }
        )
    return maps


def _postprocess(results, x_q):
    y = np.empty((B, S, H), np.float32)
    for b in range(B):
        acc = results[2 * b]["yT"].astype(np.float32) + results[2 * b + 1][
            "yT"
        ].astype(np.float32)
        y[b] = x_q[b].astype(np.float32) + acc.T
    return y


def kernel(x_q, x_kv, bias, Wq, Wk, Wv, Wo):
    x_q = np.asarray(x_q)
    nc = _build()
    maps = _in_maps(x_q, np.asarray(x_kv), np.asarray(bias), np.asarray(Wq),
                    np.asarray(Wk), np.asarray(Wv), np.asarray(Wo))
    from concourse.bass_utils import run_bass_kernel_spmd

    res = run_bass_kernel_spmd(nc, maps, list(range(N_CORES)))
    return _postprocess(res.results, x_q)
